# revision 2
# baseline (speedup 1.0000x reference)
"""W4A16 quant linear, mixed fp8/fp16 K-split on 8 trn2 NeuronCores.

Column-parallel: qweight/scales/zeros sharded along out_features, x replicated.
y = (x @ blockdiag(R_in)) @ ((q - z) * s).T   (perm == arange fast path)

Per core, the K=4096 contraction is split:
  - k in [0, 2048): fp8_e4m3 DoubleRow GEMM at 2x rate. Weights (q-8) are
    EXACT in e4m3; activations are rotated on the PE in fp16 (stationary
    block-diagonal B tile) then single-cast to e4m3 (the only quantization
    error, ~2.65% on this half of K -> ~1.87% of y).
  - k in [2048, 4096): fp16 GEMM with the rotation folded into the weights
    (baseline scheme): G16 = Bg @ (q-8)^T computed once on-device, unscaled.
Both halves accumulate into the same PSUM; drain applies the per-out-feature
scale s (and the rowsum zero-point correction when z != 8, via an extra
all-ones weight column in both halves).
"""

import numpy as np

M, K, N = 8192, 4096, 11008
NCORES = 8
NS = N // NCORES  # 1376
KT = 32  # total 128-k tiles
F8T = 16  # k-tiles in fp8 (k < 2048)
SLABS = F8T // 2  # 8 DoubleRow slabs
W16T = KT - F8T  # 16 k-tiles in fp16
MT = M // 128  # 64
WCHUNK = 4  # k-tiles per fp16 weight-prep chunk


def _body(tc, x, bta, btw, g8, qt8, scales, zeros, y, mt, zcorr):
    import concourse.mybir as mybir

    nc = tc.nc
    fp16 = mybir.dt.float16
    fp32 = mybir.dt.float32
    fp8 = mybir.dt.float8e4
    u8 = mybir.dt.uint8
    sub = mybir.AluOpType.subtract
    mult = mybir.AluOpType.mult
    DR = mybir.MatmulPerfMode.DoubleRow

    ns1 = NS + 1 if zcorr else NS
    # psum slices (offset, width); matmul chunks == slices (<=512)
    PS = [(0, 512), (512, 512), (1024, ns1 - 1024)]

    with (
        tc.tile_pool(name="g8pool", bufs=1) as g8pool,
        tc.tile_pool(name="g16pool", bufs=1) as g16pool,
        tc.tile_pool(name="btpool", bufs=1) as btpool,
        tc.tile_pool(name="szpool", bufs=1) as szpool,
        tc.tile_pool(name="qstage", bufs=2) as qpool,
        tc.tile_pool(name="xt", bufs=3) as xtpool,
        tc.tile_pool(name="hi", bufs=2) as hipool,
        tc.tile_pool(name="yout", bufs=3) as ypool,
        tc.tile_pool(name="rpsum", bufs=2, space="PSUM") as rpsum,
        tc.tile_pool(name="dpsum", bufs=2, space="PSUM") as dpsum,
    ):
        G8 = g8pool.tile([128, F8T, ns1], fp8)
        nc.gpsimd.dma_start(out=G8[:], in_=g8[:].rearrange("(t p) n -> p t n", p=128))
        BTA = btpool.tile([128, F8T, 128], fp16)
        nc.gpsimd.dma_start(out=BTA[:], in_=bta[:])
        BTW = btpool.tile([128, W16T, 128], fp16)
        nc.gpsimd.dma_start(out=BTW[:], in_=btw[:])

        s_rep = szpool.tile([128, NS], fp16)
        nc.gpsimd.dma_start(
            out=s_rep[:],
            in_=scales[:].rearrange("n o -> o n").to_broadcast([128, NS]),
        )
        if zcorr:
            z_rep = szpool.tile([128, NS], fp16)
            nc.gpsimd.dma_start(
                out=z_rep[:],
                in_=zeros[:].rearrange("n o -> o n").to_broadcast([128, NS]),
            )
            zs_rep = szpool.tile([128, NS], fp16)
            nc.vector.tensor_scalar(
                out=zs_rep[:], in0=z_rep[:], scalar1=8.0, scalar2=None, op0=sub
            )
            nc.vector.tensor_tensor(zs_rep[:], zs_rep[:], s_rep[:], mult)

        # ---- fp16 weight pipeline: G16 = Bg @ (q-8)^T, unscaled ----------
        G16 = g16pool.tile([128, W16T, ns1], fp16)
        for c0 in range(0, W16T, WCHUNK):
            qtile = qpool.tile([128, WCHUNK, NS], u8, tag="q")
            nc.gpsimd.dma_start(
                out=qtile[:],
                in_=qt8[c0 * 128 : (c0 + WCHUNK) * 128, :].rearrange(
                    "(s p) n -> p s n", p=128
                ),
            )
            wdt = qpool.tile([128, WCHUNK, ns1], fp16, tag="w")
            if zcorr:
                nc.vector.memset(wdt[:, :, NS:], 1.0)
            nc.vector.tensor_scalar(
                out=wdt[:, :, :NS], in0=qtile[:], scalar1=8.0, scalar2=None, op0=sub
            )
            for gl in range(WCHUNK):
                g = c0 + gl
                for si, (off, w) in enumerate(PS):
                    ps = dpsum.tile([128, 512], fp32, tag="py0")
                    nc.tensor.matmul(
                        ps[:, :w],
                        BTW[:, g, :],
                        wdt[:, gl, off : off + w],
                        start=True,
                        stop=True,
                    )
                    if si == 1:
                        nc.vector.tensor_copy(G16[:, g, off : off + w], ps[:, :w])
                    else:
                        nc.scalar.copy(G16[:, g, off : off + w], ps[:, :w])

        def load_xt(m):
            xt = xtpool.tile([128, KT, 128], fp16, tag="xt")
            nc.sync.dma_start(
                out=xt[:], in_=x[m * 128 : (m + 1) * 128, :], transpose=True
            )
            return xt

        def rot_group(xt, hi8, tg):
            # 4 rotation matmuls into one psum bank (later group members
            # overwrite their own untouched quarters), then a single 512-wide
            # cast -- 4x fewer cast instructions pacing the PE queue
            rps = rpsum.tile([128, 4, 128], fp32, tag="rot")
            for j in range(4):
                t = 4 * tg + j
                nc.tensor.matmul(
                    rps[:, j, :],
                    BTA[:, t, :],
                    xt[:, t, :],
                    start=(j == 0),
                    stop=(j == 3),
                    skip_group_check=True,
                )
            if tg % 2 == 0:
                nc.scalar.copy(hi8[:, 4 * tg : 4 * tg + 4, :], rps[:])
            else:
                nc.vector.tensor_copy(hi8[:, 4 * tg : 4 * tg + 4, :], rps[:])

        import contextlib

        def gemm_m(pys, hi8, xt, interleave_rot=None, mtag=None):
            # fp8 slabs (k < 2048), with next tile's rotations interleaved
            cm = nc.named_scope(f"f8_{mtag}") if mtag is not None else contextlib.nullcontext()
            with cm:
                for s in range(SLABS):
                    if interleave_rot is not None and s % 2 == 0:
                        xt_n, hi_n = interleave_rot
                        rot_group(xt_n, hi_n, s // 2)
                    for si, (off, w) in enumerate(PS):
                        nc.tensor.matmul(
                            pys[si][:, :w],
                            hi8[:, 2 * s : 2 * s + 2, :],
                            G8[:, 2 * s : 2 * s + 2, off : off + w],
                            start=(s == 0),
                            stop=False,
                            perf_mode=DR,
                        )
            # fp16 k-tiles (k >= 2048)
            cm = nc.named_scope(f"f16_{mtag}") if mtag is not None else contextlib.nullcontext()
            with cm:
                for g in range(W16T):
                    for si, (off, w) in enumerate(PS):
                        nc.tensor.matmul(
                            pys[si][:, :w],
                            xt[:, F8T + g, :],
                            G16[:, g, off : off + w],
                            start=False,
                            stop=(g == W16T - 1),
                        )

        # ---- prologue: first x tile + its rotation ------------------------
        xt_cur = load_xt(0)
        hi_cur = hipool.tile([128, F8T, 128], fp8, tag="hi")
        for tg in range(F8T // 4):
            rot_group(xt_cur, hi_cur, tg)
        if mt > 1:
            xt_nxt = load_xt(1)

        for m in range(mt):
            py0 = dpsum.tile([128, PS[0][1]], fp32, tag="py0")
            py1 = dpsum.tile([128, PS[1][1]], fp32, tag="py1")
            py2 = dpsum.tile([128, PS[2][1]], fp32, tag="py2")
            pys = [py0, py1, py2]
            if m + 1 < mt:
                hi_nxt = hipool.tile([128, F8T, 128], fp8, tag="hi")
                if m + 2 < mt:
                    xt_fut = load_xt(m + 2)
                gemm_m(pys, hi_cur, xt_cur, interleave_rot=(xt_nxt, hi_nxt), mtag=(m if m in (10, 30, 50) else None))
            else:
                gemm_m(pys, hi_cur, xt_cur)

            yt = ypool.tile([128, NS], fp16, tag="y")
            if zcorr:
                scol = ypool.tile([128, 1], fp32, tag="scol")
                nc.vector.tensor_copy(scol[:], py2[:, NS - 1024 : NS - 1024 + 1])
                tzs = ypool.tile([128, NS], fp16, tag="tzs")
                nc.vector.tensor_scalar(
                    out=tzs[:], in0=zs_rep[:], scalar1=scol[:], scalar2=None, op0=mult
                )
            nc.vector.tensor_tensor(yt[:, 0:512], py0[:], s_rep[:, 0:512], mult)
            nc.vector.tensor_tensor(yt[:, 512:1024], py1[:], s_rep[:, 512:1024], mult)
            nc.vector.tensor_tensor(
                yt[:, 1024:NS], py2[:, : NS - 1024], s_rep[:, 1024:NS], mult
            )
            if zcorr:
                nc.gpsimd.tensor_tensor(yt[:], yt[:], tzs[:], sub)
            nc.scalar.dma_start(out=y[m * 128 : (m + 1) * 128, :], in_=yt[:])

            if m + 1 < mt:
                xt_cur, hi_cur = xt_nxt, hi_nxt
                if m + 2 < mt:
                    xt_nxt = xt_fut


_CACHE = {}


def build(mt=MT, zcorr=False):
    if (mt, zcorr) in _CACHE:
        return _CACHE[(mt, zcorr)]
    import concourse.mybir as mybir
    import concourse.tile as tile
    from concourse import bacc

    fp16 = mybir.dt.float16
    fp8 = mybir.dt.float8e4
    u8 = mybir.dt.uint8
    ns1 = NS + 1 if zcorr else NS

    nc = bacc.Bacc("TRN2", target_bir_lowering=False, debug=False, num_devices=NCORES)
    x = nc.dram_tensor("x", [mt * 128, K], fp16, kind="ExternalInput")
    bta = nc.dram_tensor("bta", [128, F8T, 128], fp16, kind="ExternalInput")
    btw = nc.dram_tensor("btw", [128, W16T, 128], fp16, kind="ExternalInput")
    g8 = nc.dram_tensor("g8", [F8T * 128, ns1], fp8, kind="ExternalInput")
    qt8 = nc.dram_tensor("qt8", [W16T * 128, NS], u8, kind="ExternalInput")
    scales = nc.dram_tensor("scales", [NS, 1], fp16, kind="ExternalInput")
    zeros = nc.dram_tensor("zeros", [NS, 1], fp16, kind="ExternalInput")
    y = nc.dram_tensor("y", [mt * 128, NS], fp16, kind="ExternalOutput")

    with tile.TileContext(nc) as tc:
        _body(tc, x, bta, btw, g8, qt8, scales, zeros, y, mt, zcorr)
    nc.compile()
    _CACHE[(mt, zcorr)] = nc
    return nc


def _build_bta(rin):
    """Activation-rotation tiles, k-tiles [0, F8T): Bt[p, g, j] = B_g[p, j]."""
    bt = np.zeros((F8T, 128, 128), dtype=np.float16)
    for b in range(F8T * 8):
        g, h = divmod(b, 8)
        bt[g, h * 16 : (h + 1) * 16, h * 16 : (h + 1) * 16] = rin[b]
    return np.ascontiguousarray(bt.transpose(1, 0, 2))


def _build_btw(rin):
    """Weight-rotation tiles (transposed), k-tiles [F8T, KT)."""
    bt = np.zeros((W16T, 128, 128), dtype=np.float16)
    for b in range(F8T * 8, KT * 8):
        g, h = divmod(b - F8T * 8, 8)
        bt[g, h * 16 : (h + 1) * 16, h * 16 : (h + 1) * 16] = rin[b].T
    return np.ascontiguousarray(bt.transpose(1, 0, 2))


def run(inputs, mt=MT, trace=False):
    import ml_dtypes
    from concourse.bass_utils import run_bass_kernel_spmd

    x = np.ascontiguousarray(inputs["x"], dtype=np.float16)
    rin = np.ascontiguousarray(inputs["R_in"], dtype=np.float16)
    scales = np.ascontiguousarray(inputs["scales"], dtype=np.float16)
    zeros = np.ascontiguousarray(inputs["zeros"], dtype=np.float16)
    perm = np.asarray(inputs["perm"])
    qw = np.asarray(inputs["qweight"])

    if not np.array_equal(perm, np.arange(K, dtype=perm.dtype)):
        x = np.ascontiguousarray(x[:, perm])

    zcorr = not np.all(zeros == np.float16(8.0))
    ns1 = NS + 1 if zcorr else NS

    bta = _build_bta(rin)
    btw = _build_btw(rin)
    # fp8 half: lossless re-encode of (q-8); fp16 half: raw uint4 codes
    w8t = (qw[:, : F8T * 128].astype(np.int8) - 8).astype(ml_dtypes.float8_e4m3).T
    qu8t = qw[:, F8T * 128 :].astype(np.uint8).T

    nc = build(mt, zcorr)
    in_maps = []
    for i in range(NCORES):
        sl = slice(i * NS, (i + 1) * NS)
        g8 = np.empty((F8T * 128, ns1), dtype=ml_dtypes.float8_e4m3)
        g8[:, :NS] = w8t[:, sl]
        if zcorr:
            g8[:, NS] = ml_dtypes.float8_e4m3(1.0)
        in_maps.append(
            {
                "x": x[: mt * 128],
                "bta": bta,
                "btw": btw,
                "g8": g8,
                "qt8": np.ascontiguousarray(qu8t[:, sl]),
                "scales": scales[sl],
                "zeros": zeros[sl],
            }
        )
    res = run_bass_kernel_spmd(nc, in_maps, core_ids=list(range(NCORES)), trace=trace)
    yfull = np.concatenate([res.results[i]["y"] for i in range(NCORES)], axis=1)
    return yfull, res


def kernel(**inputs) -> np.ndarray:
    y, _ = run(inputs)
    return y
